# revision 8
# baseline (speedup 1.0000x reference)
"""Trainium2 Bass kernel for nn_MetaLayer_2551210573871 (dense_mlp).

Math:  out[b,o] = sum_i feature[b,i] * ((signal @ T_1).reshape(B,I,O)[b,i,o] + M_1[i,o])
             = sum_{s,i} signal[b,s]*feature[b,i]*T_1[s,i,o]  +  (feature @ M_1)[b,o]

Strategy (data-parallel over batch, 8 cores, B_local = 512):
  For each s-pair p: G = feature_local @ T_1[2p:2p+2]  (PE, bf16 operands)
  out_local = sum_s signal_local[:, s] * G_s + feature_local @ M_1

vs the f32r baseline:
  - all matmul operands bf16 (same 1 cyc/col PE rate as f32r, half DMA/SBUF;
    LDWEIGHTS ~104ns vs 173, hides behind 227ns matmuls)
  - featT/T1 layouts prepped on host (no on-chip transposes); full T1 resident
    in SBUF (16.8 MB), DMA'd once in use order
  - stage B path mix from measured in-situ costs, pair-outer/bt-inner keeps 4
    independent accumulation chains per engine:
      path i  : 2x DVE scalar_tensor_tensor f32 PSUM -> accA/accB[bt]
      path ii : 2x ACT scaled-copy ->bf16 tmp + GPS bf16 512-add -> subG[bt]
      path iii: 2x ACT scaled-copy ->bf16 tmp + DVE bf16 512-add -> subV[bt]
    bf16 subaccs flushed to f32 accs every FLUSH_EVERY path-pairs (bounds the
    bf16 accumulation error ~4e-3, far under the 2e-2 gate); first write
    after each flush is a copy, so no memsets in the steady state.
"""
import numpy as np
import ml_dtypes

import concourse.bacc as bacc
import concourse.mybir as mybir
import concourse.tile as tile
from concourse.bass_utils import run_bass_kernel_spmd

S_DIM, IN_DIM, OUT_DIM, BATCH = 128, 256, 256, 4096
N_CORES = 8
BL = BATCH // N_CORES          # 512 examples per core
NBT = BL // 128                # 4 batch tiles of 128
NPAIR = S_DIM // 2             # 64 s-pairs
FLUSH_EVERY = 32               # flush period per sub-accumulator (in ii-pairs)

F32 = mybir.dt.float32
BF16 = mybir.dt.bfloat16
ALU = mybir.AluOpType
ACTF = mybir.ActivationFunctionType

# stage-B path fractions per (pair,bt) unit, from measured in-situ costs:
#   i: DVE 2x394   ii: ACT 2x488 + half a GPS 1024-wide add (~1500/2 pairs)
_PATH_FRACS = {"i": 0.55, "ii": 0.45}


def _make_assignment(n_units):
    n_tail = 8 * NBT               # last 8 pairs: all path-i (GPS drains early)
    n_main = n_units - n_tail
    target_ii = _PATH_FRACS["ii"] * n_units
    frac_ii_main = target_ii / n_main
    out = []
    used = 0.0
    for u in range(n_main):
        if (u + 1) * frac_ii_main - used >= 1.0:
            out.append("ii")
            used += 1
        else:
            out.append("i")
    out.extend(["i"] * n_tail)
    return out


def _build():
    nc = bacc.Bacc("TRN2", target_bir_lowering=False, debug=False, num_devices=N_CORES)

    sig_d = nc.dram_tensor("sig", [BL, S_DIM], F32, kind="ExternalInput")
    featT_d = nc.dram_tensor("featT", [128, 2 * BL], BF16, kind="ExternalInput")
    t1_d = nc.dram_tensor("t1", [128, 2 * NPAIR * 512], BF16, kind="ExternalInput")
    m1_d = nc.dram_tensor("m1", [IN_DIM, OUT_DIM], BF16, kind="ExternalInput")
    out_d = nc.dram_tensor("out", [BL, OUT_DIM], F32, kind="ExternalOutput")

    with tile.TileContext(nc) as tc:
        assignment = _make_assignment(NPAIR * NBT)
        with (
            tc.tile_pool(name="const", bufs=1) as const,
            tc.tile_pool(name="tmp", bufs=8) as tmp_pool,
            tc.tile_pool(name="psum", bufs=8, space="PSUM") as psum,
        ):
            # --- persistent inputs (DMA in use order) ---
            featT = const.tile([128, 2 * BL], BF16, tag="featT", name="featT")
            nc.sync.dma_start(out=featT[:], in_=featT_d[:, :])

            m1 = []
            for ic in range(2):
                t = const.tile([128, OUT_DIM], BF16, tag=f"m1_{ic}", name=f"m1_{ic}")
                nc.sync.dma_start(out=t[:], in_=m1_d[ic * 128:(ic + 1) * 128, :])
                m1.append(t)

            def load_t1(p, t1t):
                for ic in range(2):
                    t = const.tile([128, 512], BF16, tag=f"t1_{ic}_{p}",
                                   name=f"t1_{ic}_{p}")
                    base = (ic * NPAIR + p) * 512
                    nc.sync.dma_start(out=t[:], in_=t1_d[:, base:base + 512])
                    t1t[(ic, p)] = t

            sig = []
            for bt in range(NBT):
                t = const.tile([128, S_DIM], F32, tag=f"sig{bt}", name=f"sig{bt}")
                nc.sync.dma_start(out=t[:], in_=sig_d[bt * 128:(bt + 1) * 128, :])
                sig.append(t)

            t1t = {}
            for p in range(NPAIR):
                load_t1(p, t1t)

            # preload ACT function table while DMAs stream
            warm = const.tile([128, 8], F32, tag="warm", name="warm")
            nc.scalar.activation(warm[:], sig[0][:, 0:8], ACTF.Copy,
                                 bias=0.0, scale=1.0)

            # --- accumulators ---
            accA, accB, accC, accD, subG = [], [], [], [], []
            for bt in range(NBT):
                accA.append(const.tile([128, OUT_DIM], F32, tag=f"accA{bt}",
                                       name=f"accA{bt}"))
                accB.append(const.tile([128, OUT_DIM], F32, tag=f"accB{bt}",
                                       name=f"accB{bt}"))
                accC.append(const.tile([128, OUT_DIM], F32, tag=f"accC{bt}",
                                       name=f"accC{bt}"))
                accD.append(const.tile([128, OUT_DIM], F32, tag=f"accD{bt}",
                                       name=f"accD{bt}"))
                subG.append(const.tile([128, 1024], BF16, tag=f"subG{bt}",
                                       name=f"subG{bt}"))

            # accA[bt] = feature @ M_1 ; accB[bt] = 0
            for bt in range(NBT):
                ps = psum.tile([128, 512], F32, tag="G", name="ps_init")
                for ic in range(2):
                    nc.tensor.matmul(
                        ps[:, 0:OUT_DIM],
                        featT[:, ic * BL + bt * 128:ic * BL + (bt + 1) * 128],
                        m1[ic][:],
                        start=(ic == 0),
                        stop=(ic == 1),
                    )
                nc.vector.tensor_copy(accA[bt][:], ps[:, 0:OUT_DIM])
                nc.gpsimd.memset(accB[bt][:], 0)
                nc.gpsimd.memset(accC[bt][:], 0)
                nc.gpsimd.memset(accD[bt][:], 0)
                nc.gpsimd.memset(subG[bt][:], 0)

            subG_n = [0] * NBT    # ii-pairs since last flush (0 => fresh)
            pend = [None] * NBT   # pending half-filled double-tmp tile

            def flush(bt, reset=True):
                for q in range(4):
                    acc = accC[bt] if q % 2 == 0 else accD[bt]
                    nc.vector.tensor_tensor(
                        acc[:], subG[bt][:, q * 256:q * 256 + 256],
                        acc[:], ALU.add)
                if reset:
                    nc.gpsimd.memset(subG[bt][:], 0)
                subG_n[bt] = 0

            # --- main loop: pair-outer, bt-inner ---
            for p in range(NPAIR):
                g = [psum.tile([128, 512], F32, tag="G", name=f"g{bt}")
                     for bt in range(NBT)]
                for ic in range(2):
                    for bt in range(NBT):
                        nc.tensor.matmul(
                            g[bt][:],
                            featT[:, ic * BL + bt * 128:ic * BL + (bt + 1) * 128],
                            t1t[(ic, p)][:],
                            start=(ic == 0), stop=(ic == 1),
                        )
                s0, s1 = 2 * p, 2 * p + 1
                for bt in range(NBT):
                    mode = assignment[p * NBT + bt]
                    if mode == "i":
                        acc = accA[bt] if p % 2 == 0 else accB[bt]
                        for half, s in ((0, s0), (1, s1)):
                            nc.vector.scalar_tensor_tensor(
                                acc[:], g[bt][:, half * 256:half * 256 + 256],
                                sig[bt][:, s:s + 1], acc[:],
                                ALU.mult, ALU.add,
                            )
                    else:
                        if pend[bt] is None:
                            pend[bt] = tmp_pool.tile([128, 1024], BF16,
                                                     tag="tmp", name="tmp")
                            off = 0
                        else:
                            off = 512
                        tmp = pend[bt]
                        for half, s in ((0, s0), (1, s1)):
                            nc.scalar.activation(
                                tmp[:, off + half * 256:off + half * 256 + 256],
                                g[bt][:, half * 256:half * 256 + 256],
                                ACTF.Copy, bias=0.0,
                                scale=sig[bt][:, s:s + 1],
                            )
                        if off == 512:
                            nc.gpsimd.tensor_tensor(
                                subG[bt][:], tmp[:], subG[bt][:], ALU.add)
                            pend[bt] = None
                            subG_n[bt] += 2
                            if subG_n[bt] >= FLUSH_EVERY:
                                flush(bt)

            # --- final flush + merge + store ---
            for bt in range(NBT):
                if pend[bt] is not None:
                    # leftover half-filled double-tmp: 512-wide add
                    nc.gpsimd.tensor_tensor(
                        subG[bt][:, 0:512], pend[bt][:, 0:512],
                        subG[bt][:, 0:512], ALU.add)
                    pend[bt] = None
                    subG_n[bt] += 1
                if subG_n[bt]:
                    flush(bt, reset=False)
                nc.vector.tensor_tensor(accC[bt][:], accD[bt][:], accC[bt][:],
                                        ALU.add)
                nc.vector.tensor_tensor(accB[bt][:], accC[bt][:], accB[bt][:],
                                        ALU.add)
                nc.vector.tensor_tensor(accA[bt][:], accB[bt][:], accA[bt][:],
                                        ALU.add)
                nc.sync.dma_start(
                    out=out_d[bt * 128:(bt + 1) * 128, :], in_=accA[bt][:]
                )

    nc.compile()
    return nc


_cached = None


def make_in_maps(signal, feature, T_1, M_1):
    signal = np.ascontiguousarray(np.asarray(signal, dtype=np.float32))
    feature = np.asarray(feature, dtype=np.float32)
    M_1bf = np.ascontiguousarray(
        np.asarray(M_1, dtype=np.float32).astype(ml_dtypes.bfloat16))
    # T1 [s,i,o] -> [k, ic, p, half, o]: tile (ic,p) = [128, 512] with cols
    # [s=2p: o | s=2p+1: o], bf16
    T1bf = np.ascontiguousarray(
        np.asarray(T_1, dtype=np.float32)
        .reshape(NPAIR, 2, 2, 128, OUT_DIM)       # [p, half, ic, k, o]
        .transpose(3, 2, 0, 1, 4)                 # [k, ic, p, half, o]
        .reshape(128, 2 * NPAIR * 512)
        .astype(ml_dtypes.bfloat16))
    in_maps = []
    for c in range(N_CORES):
        sl = slice(c * BL, (c + 1) * BL)
        feat_l = feature[sl]                      # [BL, 256]
        featT = np.ascontiguousarray(
            feat_l.T.reshape(2, 128, BL)          # [ic, k, b]
            .transpose(1, 0, 2)                   # [k, ic, b]
            .reshape(128, 2 * BL)
            .astype(ml_dtypes.bfloat16))
        in_maps.append({
            "sig": signal[sl],
            "featT": featT,
            "t1": T1bf,
            "m1": M_1bf,
        })
    return in_maps


def kernel(signal, feature, T_1, M_1):
    global _cached
    if _cached is None:
        _cached = _build()
    nc = _cached
    in_maps = make_in_maps(signal, feature, T_1, M_1)
    res = run_bass_kernel_spmd(nc, in_maps, list(range(N_CORES))).results
    return np.concatenate([res[c]["out"] for c in range(N_CORES)], axis=0)


# revision 9
# speedup vs baseline: 1.1282x; 1.1282x over previous
"""Trainium2 Bass kernel for nn_MetaLayer_2551210573871 (dense_mlp).

Math:  out[b,o] = sum_i feature[b,i] * ((signal @ T_1).reshape(B,I,O)[b,i,o] + M_1[i,o])
             = sum_{s,i} signal[b,s]*feature[b,i]*T_1[s,i,o]  +  (feature @ M_1)[b,o]

Strategy (data-parallel over batch, 8 cores, B_local = 512):
  For each s-pair p: G = feature_local @ T_1[2p:2p+2]  (PE, bf16 operands)
  out_local = sum_s signal_local[:, s] * G_s + feature_local @ M_1

vs the f32r baseline:
  - all matmul operands bf16 (same 1 cyc/col PE rate as f32r, half DMA/SBUF;
    LDWEIGHTS ~104ns vs 173, hides behind 227ns matmuls)
  - featT/T1 layouts prepped on host (no on-chip transposes); full T1 resident
    in SBUF (16.8 MB), DMA'd once in use order
  - stage B path mix from measured in-situ costs, pair-outer/bt-inner keeps 4
    independent accumulation chains per engine:
      path i  : 2x DVE scalar_tensor_tensor f32 PSUM -> accA/accB[bt]
      path ii : 2x ACT scaled-copy ->bf16 tmp + GPS bf16 512-add -> subG[bt]
      path iii: 2x ACT scaled-copy ->bf16 tmp + DVE bf16 512-add -> subV[bt]
    bf16 subaccs flushed to f32 accs every FLUSH_EVERY path-pairs (bounds the
    bf16 accumulation error ~4e-3, far under the 2e-2 gate); first write
    after each flush is a copy, so no memsets in the steady state.
"""
import numpy as np
import ml_dtypes

import concourse.bacc as bacc
import concourse.mybir as mybir
import concourse.tile as tile
from concourse.bass_utils import run_bass_kernel_spmd

S_DIM, IN_DIM, OUT_DIM, BATCH = 128, 256, 256, 4096
N_CORES = 8
BL = BATCH // N_CORES          # 512 examples per core
NBT = BL // 128                # 4 batch tiles of 128
NPAIR = S_DIM // 2             # 64 s-pairs
FLUSH_EVERY = 32               # flush period per sub-accumulator (in ii-pairs)

F32 = mybir.dt.float32
BF16 = mybir.dt.bfloat16
ALU = mybir.AluOpType
ACTF = mybir.ActivationFunctionType

# stage-B path fractions per (pair,bt) unit, from measured in-situ costs:
#   i: DVE 2x394   ii: ACT 2x488 + half a GPS 1024-wide add (~1500/2 pairs)
_PATH_FRACS = {"i": 0.55, "ii": 0.45}


def _make_assignment(n_units):
    used = {k: 0.0 for k in _PATH_FRACS}
    out = []
    for u in range(n_units):
        best, best_def = None, None
        for k, f in _PATH_FRACS.items():
            deficit = f * (u + 1) - used[k]
            if best_def is None or deficit > best_def:
                best, best_def = k, deficit
        used[best] += 1
        out.append(best)
    # last 4 pairs: path-i only, so GPS/flush chains drain before the tail
    for u in range(n_units - 4 * NBT, n_units):
        out[u] = "i"
    return out


def _build():
    nc = bacc.Bacc("TRN2", target_bir_lowering=False, debug=False, num_devices=N_CORES)

    sig_d = nc.dram_tensor("sig", [BL, S_DIM], F32, kind="ExternalInput")
    featT_d = nc.dram_tensor("featT", [128, 2 * BL], BF16, kind="ExternalInput")
    t1_d = nc.dram_tensor("t1", [128, 2 * NPAIR * 512], BF16, kind="ExternalInput")
    m1_d = nc.dram_tensor("m1", [IN_DIM, OUT_DIM], BF16, kind="ExternalInput")
    out_d = nc.dram_tensor("out", [BL, OUT_DIM], F32, kind="ExternalOutput")

    with tile.TileContext(nc) as tc:
        assignment = _make_assignment(NPAIR * NBT)
        with (
            tc.tile_pool(name="const", bufs=1) as const,
            tc.tile_pool(name="tmp", bufs=8) as tmp_pool,
            tc.tile_pool(name="psum", bufs=8, space="PSUM") as psum,
        ):
            # --- persistent inputs (DMA in use order) ---
            featT = const.tile([128, 2 * BL], BF16, tag="featT", name="featT")
            nc.sync.dma_start(out=featT[:], in_=featT_d[:, :])

            m1 = []
            for ic in range(2):
                t = const.tile([128, OUT_DIM], BF16, tag=f"m1_{ic}", name=f"m1_{ic}")
                nc.sync.dma_start(out=t[:], in_=m1_d[ic * 128:(ic + 1) * 128, :])
                m1.append(t)

            def load_t1(p, t1t):
                for ic in range(2):
                    t = const.tile([128, 512], BF16, tag=f"t1_{ic}_{p}",
                                   name=f"t1_{ic}_{p}")
                    base = (ic * NPAIR + p) * 512
                    nc.sync.dma_start(out=t[:], in_=t1_d[:, base:base + 512])
                    t1t[(ic, p)] = t

            sig = []
            for bt in range(NBT):
                t = const.tile([128, S_DIM], F32, tag=f"sig{bt}", name=f"sig{bt}")
                nc.sync.dma_start(out=t[:], in_=sig_d[bt * 128:(bt + 1) * 128, :])
                sig.append(t)

            t1t = {}
            for p in range(NPAIR):
                load_t1(p, t1t)

            # preload ACT function table while DMAs stream
            warm = const.tile([128, 8], F32, tag="warm", name="warm")
            nc.scalar.activation(warm[:], sig[0][:, 0:8], ACTF.Copy,
                                 bias=0.0, scale=1.0)

            # --- accumulators ---
            accA, accB, accC, accD, subG = [], [], [], [], []
            for bt in range(NBT):
                accA.append(const.tile([128, OUT_DIM], F32, tag=f"accA{bt}",
                                       name=f"accA{bt}"))
                accB.append(const.tile([128, OUT_DIM], F32, tag=f"accB{bt}",
                                       name=f"accB{bt}"))
                accC.append(const.tile([128, OUT_DIM], F32, tag=f"accC{bt}",
                                       name=f"accC{bt}"))
                accD.append(const.tile([128, OUT_DIM], F32, tag=f"accD{bt}",
                                       name=f"accD{bt}"))
                subG.append(const.tile([128, 1024], BF16, tag=f"subG{bt}",
                                       name=f"subG{bt}"))

            # accA[bt] = feature @ M_1 ; accB[bt] = 0
            for bt in range(NBT):
                ps = psum.tile([128, 512], F32, tag="G", name="ps_init")
                for ic in range(2):
                    nc.tensor.matmul(
                        ps[:, 0:OUT_DIM],
                        featT[:, ic * BL + bt * 128:ic * BL + (bt + 1) * 128],
                        m1[ic][:],
                        start=(ic == 0),
                        stop=(ic == 1),
                    )
                nc.vector.tensor_copy(accA[bt][:], ps[:, 0:OUT_DIM])
                nc.gpsimd.memset(accB[bt][:], 0)
                nc.gpsimd.memset(accC[bt][:], 0)
                nc.gpsimd.memset(accD[bt][:], 0)
                nc.gpsimd.memset(subG[bt][:], 0)

            subG_n = [0] * NBT    # ii-pairs since last flush (0 => fresh)
            pend = [None] * NBT   # pending half-filled double-tmp tile

            def flush(bt, reset=True):
                for q in range(4):
                    acc = accC[bt] if q % 2 == 0 else accD[bt]
                    nc.vector.tensor_tensor(
                        acc[:], subG[bt][:, q * 256:q * 256 + 256],
                        acc[:], ALU.add)
                if reset:
                    nc.gpsimd.memset(subG[bt][:], 0)
                subG_n[bt] = 0

            # --- main loop: pair-outer, bt-inner ---
            for p in range(NPAIR):
                g = [psum.tile([128, 512], F32, tag="G", name=f"g{bt}")
                     for bt in range(NBT)]
                for ic in range(2):
                    for bt in range(NBT):
                        nc.tensor.matmul(
                            g[bt][:],
                            featT[:, ic * BL + bt * 128:ic * BL + (bt + 1) * 128],
                            t1t[(ic, p)][:],
                            start=(ic == 0), stop=(ic == 1),
                        )
                s0, s1 = 2 * p, 2 * p + 1
                for bt in range(NBT):
                    mode = assignment[p * NBT + bt]
                    if mode == "i":
                        acc = accA[bt] if p % 2 == 0 else accB[bt]
                        for half, s in ((0, s0), (1, s1)):
                            nc.vector.scalar_tensor_tensor(
                                acc[:], g[bt][:, half * 256:half * 256 + 256],
                                sig[bt][:, s:s + 1], acc[:],
                                ALU.mult, ALU.add,
                            )
                    else:
                        if pend[bt] is None:
                            pend[bt] = tmp_pool.tile([128, 1024], BF16,
                                                     tag="tmp", name="tmp")
                            off = 0
                        else:
                            off = 512
                        tmp = pend[bt]
                        for half, s in ((0, s0), (1, s1)):
                            nc.scalar.activation(
                                tmp[:, off + half * 256:off + half * 256 + 256],
                                g[bt][:, half * 256:half * 256 + 256],
                                ACTF.Copy, bias=0.0,
                                scale=sig[bt][:, s:s + 1],
                            )
                        if off == 512:
                            nc.gpsimd.tensor_tensor(
                                subG[bt][:], tmp[:], subG[bt][:], ALU.add)
                            pend[bt] = None
                            subG_n[bt] += 2
                            if subG_n[bt] >= FLUSH_EVERY:
                                flush(bt)

            # --- final flush + merge + store ---
            for bt in range(NBT):
                if pend[bt] is not None:
                    # leftover half-filled double-tmp: 512-wide add
                    nc.gpsimd.tensor_tensor(
                        subG[bt][:, 0:512], pend[bt][:, 0:512],
                        subG[bt][:, 0:512], ALU.add)
                    pend[bt] = None
                    subG_n[bt] += 1
                if subG_n[bt]:
                    flush(bt, reset=False)
                nc.vector.tensor_tensor(accC[bt][:], accD[bt][:], accC[bt][:],
                                        ALU.add)
                nc.vector.tensor_tensor(accB[bt][:], accC[bt][:], accB[bt][:],
                                        ALU.add)
                nc.vector.tensor_tensor(accA[bt][:], accB[bt][:], accA[bt][:],
                                        ALU.add)
                nc.sync.dma_start(
                    out=out_d[bt * 128:(bt + 1) * 128, :], in_=accA[bt][:]
                )

    nc.compile()
    return nc


_cached = None


def make_in_maps(signal, feature, T_1, M_1):
    signal = np.ascontiguousarray(np.asarray(signal, dtype=np.float32))
    feature = np.asarray(feature, dtype=np.float32)
    M_1bf = np.ascontiguousarray(
        np.asarray(M_1, dtype=np.float32).astype(ml_dtypes.bfloat16))
    # T1 [s,i,o] -> [k, ic, p, half, o]: tile (ic,p) = [128, 512] with cols
    # [s=2p: o | s=2p+1: o], bf16
    T1bf = np.ascontiguousarray(
        np.asarray(T_1, dtype=np.float32)
        .reshape(NPAIR, 2, 2, 128, OUT_DIM)       # [p, half, ic, k, o]
        .transpose(3, 2, 0, 1, 4)                 # [k, ic, p, half, o]
        .reshape(128, 2 * NPAIR * 512)
        .astype(ml_dtypes.bfloat16))
    in_maps = []
    for c in range(N_CORES):
        sl = slice(c * BL, (c + 1) * BL)
        feat_l = feature[sl]                      # [BL, 256]
        featT = np.ascontiguousarray(
            feat_l.T.reshape(2, 128, BL)          # [ic, k, b]
            .transpose(1, 0, 2)                   # [k, ic, b]
            .reshape(128, 2 * BL)
            .astype(ml_dtypes.bfloat16))
        in_maps.append({
            "sig": signal[sl],
            "featT": featT,
            "t1": T1bf,
            "m1": M_1bf,
        })
    return in_maps


def kernel(signal, feature, T_1, M_1):
    global _cached
    if _cached is None:
        _cached = _build()
    nc = _cached
    in_maps = make_in_maps(signal, feature, T_1, M_1)
    res = run_bass_kernel_spmd(nc, in_maps, list(range(N_CORES))).results
    return np.concatenate([res[c]["out"] for c in range(N_CORES)], axis=0)
